# revision 1
# baseline (speedup 1.0000x reference)
"""Trainium2 Bass kernel for CausalPrefixMemory.

Computes, for x [B, S, D], W_update/W_gate [SD, D], W_out [D, SD]:
    gate = sigmoid(x @ W_gate.T); upd = x @ W_update.T
    memory = cumsum(gate * upd, axis=1)            # [B, S, SD]
    h = memory / arange(1, S+1)[None, :, None]
    h = h * rsqrt(mean(h*h, -1) + eps)             # RMSNorm, no weight
    out = h @ W_out.T                              # [B, S, D]

Sharding over 8 NeuronCores: (batch, sequence-half).  Core c < 4 handles
rows [0, S/2) of batch c; core c+4 handles rows [S/2, S) of batch c.  The
only cross-core dependency is the running prefix total of the first half
(a [SD] vector), exchanged with a 2-rank AllReduce per batch pair.

Within a core the sequence is processed in 128-row chunks:
  phase 1: x (bf16 cast-load) -> PE transpose -> gate/update projections
           (bf16 in, fp32 PSUM accum) -> gu = sigmoid(gate)*upd (fp32)
           -> running column-sum total (float32r matmul)
  carry:   masked 2-rank AllReduce of the total -> serial carry chain
  phase 2: cumsum = triangular matmul + rank-1 carry matmul (float32r,
           fp32 accum) -> fused (1/position)*rsqrt RMS scale -> PE
           transpose (bf16) -> output projection (bf16 in, fp32 accum)

All matmul moving operands are N=512 (full PE stream rate); cumsum along
the 128-row chunk is a single triangular-matrix matmul; the only
cross-chunk serialization is the [1,512] carry chain, which is ~3x
faster per chunk than the PE work it feeds.  Estimated exec time from
the calibrated CoreSim cost model: ~267us (the PE is busy ~80% of it).
"""

import sys

import numpy as np

if "/opt/trn_rl_repo" not in sys.path:
    sys.path.insert(0, "/opt/trn_rl_repo")

import concourse.bass as bass  # noqa: E402
import concourse.tile as tile  # noqa: E402
from concourse import bacc, mybir  # noqa: E402
from concourse.bass_utils import run_bass_kernel_spmd  # noqa: E402

F32 = mybir.dt.float32
F32R = mybir.dt.float32r
BF16 = mybir.dt.bfloat16

B, S, D, SD = 4, 8192, 1024, 512
N_CORES = 8
P = 128  # chunk rows == SBUF partitions
RMS_EPS = 1.1920929e-07

TRACE = False
LAST_EXEC_NS = None


def emit_core_kernel(tc, io, n_chunks, pair_groups):
    """Emit the per-core program. io maps names -> DRAM APs."""
    nc = tc.nc
    x = io["x"]            # [n_chunks*P, D]
    out = io["out"]        # [n_chunks*P, D]
    ND = D // P            # 8  d-chunks
    NK = SD // P           # 4  k-chunks

    with (
        tc.tile_pool(name="statics", bufs=1) as statics,
        tc.tile_pool(name="gu_pool", bufs=1) as gu_pool,
        tc.tile_pool(name="work", bufs=3) as work,
        tc.tile_pool(name="small", bufs=4) as small,
        tc.tile_pool(name="dram", bufs=1, space="DRAM") as dram,
    ):
        # ---- static tiles: small tables first (ident gates the very first
        # PE transpose), big weight matrices last so they don't block it ----
        identb_sb = statics.tile([P, P], BF16)
        nc.sync.dma_start(out=identb_sb, in_=io["identb"])
        ident_sb = statics.tile([P, P], F32)
        nc.sync.dma_start(out=ident_sb, in_=io["ident"])
        lcumT_sb = statics.tile([P, P], F32R)
        nc.sync.dma_start(out=lcumT_sb, in_=io["lcumT"].bitcast(F32R))
        ones_col_sb = statics.tile([P, 1], F32R)
        nc.sync.dma_start(out=ones_col_sb, in_=io["ones_col"].bitcast(F32R))
        ones_row_sb = statics.tile([1, P], F32R)
        nc.sync.dma_start(out=ones_row_sb, in_=io["ones_row"].bitcast(F32R))
        recip_sb = statics.tile([P, n_chunks], F32)
        nc.sync.dma_start(out=recip_sb, in_=io["recip"])
        r2d_sb = statics.tile([P, n_chunks], F32)
        nc.sync.dma_start(out=r2d_sb, in_=io["r2d"])
        msend_sb = statics.tile([1, 1], F32)
        nc.sync.dma_start(out=msend_sb, in_=io["mask_send"])
        mrecv_sb = statics.tile([1, 1], F32)
        nc.sync.dma_start(out=mrecv_sb, in_=io["mask_recv"])
        # weight slices interleaved in consumption order; wo (phase 2 only) last
        wg_sb = statics.tile([P, ND, SD], BF16)
        wu_sb = statics.tile([P, ND, SD], BF16)
        wgT_r = io["wgT"].rearrange("(j p) k -> p j k", p=P)
        wuT_r = io["wuT"].rearrange("(j p) k -> p j k", p=P)
        for j in range(ND):
            nc.sync.dma_start(out=wg_sb[:, j, :], in_=wgT_r[:, j, :])
            nc.sync.dma_start(out=wu_sb[:, j, :], in_=wuT_r[:, j, :])
        wo_sb = statics.tile([P, NK, D], F32R)
        nc.sync.dma_start(out=wo_sb, in_=io["woT"].bitcast(F32R).rearrange("(j p) d -> p j d", p=P))

        zero_sb = statics.tile([P, 1], F32)
        nc.vector.memset(zero_sb, 0.0)
        eps_sb = statics.tile([P, 1], F32)
        nc.vector.memset(eps_sb, RMS_EPS)

        gu_tiles = []

        # ---- phase 1: projections + gu + chunk sums -----------------------
        with (
            tc.tile_pool(name="ps_xt", bufs=3, space="PSUM") as ps_xt,
            tc.tile_pool(name="ps_gate", bufs=2, space="PSUM") as ps_gate,
            tc.tile_pool(name="ps_upd", bufs=2, space="PSUM") as ps_upd,
            tc.tile_pool(name="ps_cs", bufs=1, space="PSUM") as ps_cs,
        ):
            # running total of all chunk column sums, accumulated in one
            # PSUM row at partition 0 (arbitrary-partition APs are illegal)
            tot_ps = ps_cs.tile([1, SD], F32, tag="tot")
            for c in range(n_chunks):
                x_sb = work.tile([P, D], BF16, tag="x", name=f"x{c}")
                nc.gpsimd.dma_start(out=x_sb, in_=x[c * P : (c + 1) * P, :])

                # transpose x chunk: xt_sb[:, j*P:(j+1)*P] = x_sb[:, jP:(j+1)P].T
                xt_sb = work.tile([P, D], BF16, tag="xt", name=f"xt{c}")
                for jj in range(ND // 4):
                    pxt = ps_xt.tile([P, 4 * P], BF16, tag="pxt", name=f"pxt{c}_{jj}")
                    for j4 in range(4):
                        j = jj * 4 + j4
                        nc.tensor.transpose(
                            pxt[:, j4 * P : (j4 + 1) * P],
                            x_sb[:, j * P : (j + 1) * P],
                            identb_sb,
                        )
                    nc.vector.tensor_copy(
                        xt_sb[:, jj * 4 * P : (jj + 1) * 4 * P], pxt
                    )

                gate_ps = ps_gate.tile([P, SD], F32, tag="g", name=f"g{c}")
                upd_ps = ps_upd.tile([P, SD], F32, tag="u", name=f"u{c}")
                for j in range(ND):
                    lhsT = xt_sb[:, j * P : (j + 1) * P]
                    nc.tensor.matmul(
                        gate_ps, lhsT, wg_sb[:, j, :],
                        start=(j == 0), stop=(j == ND - 1),
                    )
                    nc.tensor.matmul(
                        upd_ps, lhsT, wu_sb[:, j, :],
                        start=(j == 0), stop=(j == ND - 1),
                    )

                sig_sb = work.tile([P, SD], F32, tag="sig", name=f"sig{c}")
                nc.scalar.activation(
                    sig_sb, gate_ps, mybir.ActivationFunctionType.Sigmoid,
                    bias=zero_sb,
                )
                gu_sb = gu_pool.tile([P, SD], F32R, tag=f"gu{c}", name=f"gu{c}")
                nc.vector.tensor_mul(gu_sb, sig_sb, upd_ps)
                gu_tiles.append(gu_sb)

                nc.tensor.matmul(
                    tot_ps, ones_col_sb, gu_sb,
                    start=(c == 0), stop=(c == n_chunks - 1),
                )

            # masked carry exchange: first-half cores contribute their total,
            # second-half cores receive it (emitted here so tot_ps is read
            # while its pool is still live)
            contrib_sb = small.tile([1, SD], F32, tag="contrib", bufs=1)
            nc.vector.tensor_scalar_mul(contrib_sb, tot_ps, msend_sb)
            cc_in = dram.tile([1, SD], F32, tag="cc_in")
            cc_out = dram.tile([1, SD], F32, tag="cc_out")
            nc.sync.dma_start(out=cc_in, in_=contrib_sb)
            nc.gpsimd.collective_compute(
                "AllReduce",
                mybir.AluOpType.add,
                replica_groups=pair_groups,
                ins=[cc_in.opt()],
                outs=[cc_out.opt()],
            )
            rraw_sb = small.tile([1, SD], F32, tag="rraw", bufs=1)
            nc.sync.dma_start(out=rraw_sb, in_=cc_out)
            r_sb = small.tile([1, SD], F32R, tag="rrow", bufs=1)
            nc.vector.tensor_scalar_mul(r_sb, rraw_sb, mrecv_sb)


        # ---- carry exchange + per-chunk carries ---------------------------
        with (
            tc.tile_pool(name="ps_cs2", bufs=1, space="PSUM") as ps_cs2,
            tc.tile_pool(name="ps_mem", bufs=3, space="PSUM") as ps_mem,
            tc.tile_pool(name="ps_ht", bufs=2, space="PSUM") as ps_ht,
            tc.tile_pool(name="ps_out", bufs=2, space="PSUM") as ps_out,
        ):
            # ---- phase 2: cumsum + normalize + output projection ----------
            # carry chain: carry_0 = R; carry_{c+1} = carry_c + colsum(gu_c)
            carry_sb = r_sb
            for c in range(n_chunks):
                if c + 1 < n_chunks:
                    cs2_ps = ps_cs2.tile([1, SD], F32, tag="cs2", name=f"cs2_{c}")
                    nc.tensor.matmul(
                        cs2_ps, ones_col_sb, gu_tiles[c], start=True, stop=True
                    )
                    carry_next = small.tile(
                        [1, SD], F32R, tag="carry", name=f"carry{c}", bufs=3
                    )
                    nc.vector.tensor_add(carry_next, carry_sb, cs2_ps)

                mem_ps = ps_mem.tile([P, SD], F32, tag="mem", name=f"mem{c}")
                nc.tensor.matmul(
                    mem_ps, lcumT_sb, gu_tiles[c], start=True, stop=False
                )
                nc.tensor.matmul(
                    mem_ps, ones_row_sb, carry_sb, start=False, stop=True
                )
                if c + 1 < n_chunks:
                    carry_sb = carry_next

                # msum = sum_k mem^2 ; scale = recip * rsqrt(msum*recip^2/SD + eps)
                sq_sb = work.tile([P, SD], F32, tag="sq", name=f"sq{c}")
                msum_sb = small.tile([P, 1], F32, tag="msum", name=f"msum{c}")
                nc.scalar.activation(
                    sq_sb, mem_ps, mybir.ActivationFunctionType.Square,
                    bias=zero_sb, accum_out=msum_sb,
                )
                sqt_sb = small.tile([P, 1], F32, tag="sqt", name=f"sqt{c}")
                nc.scalar.activation(
                    sqt_sb, msum_sb, mybir.ActivationFunctionType.Sqrt,
                    bias=eps_sb, scale=r2d_sb[:, c : c + 1],
                )
                rstd_sb = small.tile([P, 1], F32, tag="rstd", name=f"rstd{c}")
                nc.vector.reciprocal(rstd_sb, sqt_sb)
                scale_sb = small.tile([P, 1], F32, tag="scale", name=f"scale{c}")
                nc.vector.tensor_mul(scale_sb, rstd_sb, recip_sb[:, c : c + 1])

                h_sb = work.tile([P, SD], F32, tag="h", name=f"h{c}")
                nc.vector.tensor_scalar_mul(h_sb, mem_ps, scale_sb)

                ht_ps = ps_ht.tile([P, NK * P], F32, tag="ht", name=f"ht{c}")
                for j in range(NK):
                    nc.tensor.transpose(
                        ht_ps[:, j * P : (j + 1) * P],
                        h_sb[:, j * P : (j + 1) * P],
                        ident_sb,
                    )
                ht_sb = work.tile([P, NK * P], F32R, tag="hts", name=f"hts{c}")
                nc.vector.tensor_copy(ht_sb, ht_ps)

                out_sb = work.tile([P, D], F32, tag="o", name=f"o{c}")
                for half in range(2):
                    op_ps = ps_out.tile(
                        [P, D // 2], F32, tag="op", name=f"op{c}_{half}"
                    )
                    for j in range(NK):
                        nc.tensor.matmul(
                            op_ps,
                            ht_sb[:, j * P : (j + 1) * P],
                            wo_sb[:, j, half * (D // 2) : (half + 1) * (D // 2)],
                            start=(j == 0), stop=(j == NK - 1),
                        )
                    nc.scalar.copy(
                        out_sb[:, half * (D // 2) : (half + 1) * (D // 2)], op_ps
                    )
                nc.sync.dma_start(out=out[c * P : (c + 1) * P, :], in_=out_sb)


def aux_inputs(core, n_cores, s_local):
    """Per-core constant tables (host side)."""
    n_chunks = s_local // P
    first_half = core < n_cores // 2
    s0 = 0 if first_half else s_local
    pos = s0 + np.arange(n_chunks)[None, :] * P + np.arange(P)[:, None] + 1
    recip = (1.0 / pos).astype(np.float32)
    r2d = (recip * recip / np.float32(SD)).astype(np.float32)
    return {
        "recip": recip,
        "r2d": r2d,
        "mask_send": np.full((1, 1), 1.0 if first_half else 0.0, np.float32),
        "mask_recv": np.full((1, 1), 0.0 if first_half else 1.0, np.float32),
    }


def const_inputs(n_chunks):
    """Constant tables shared by all cores (host side)."""
    import ml_dtypes
    out = {
        "lcumT": np.triu(np.ones((P, P), np.float32)),        # [t,s]=1 if t<=s
        "identb": np.eye(P, dtype=np.float32),  # cast to bf16 below
        "ident": np.eye(P, dtype=np.float32),
        "ones_col": np.ones((P, 1), np.float32),
        "ones_row": np.ones((1, P), np.float32),
    }
    out["identb"] = out["identb"].astype(ml_dtypes.bfloat16)
    return out


_BUILD_CACHE = {}


def build(n_cores, s_local):
    key = (n_cores, s_local)
    if key in _BUILD_CACHE:
        return _BUILD_CACHE[key]
    n_chunks = s_local // P
    pair_groups = [[i, i + n_cores // 2] for i in range(n_cores // 2)]

    nc = bacc.Bacc(
        "TRN2", target_bir_lowering=False, debug=False, num_devices=n_cores
    )
    io = {}
    io["x"] = nc.dram_tensor("x", [s_local, D], F32, kind="ExternalInput").ap()
    io["wgT"] = nc.dram_tensor("wgT", [D, SD], BF16, kind="ExternalInput").ap()
    io["wuT"] = nc.dram_tensor("wuT", [D, SD], BF16, kind="ExternalInput").ap()
    io["woT"] = nc.dram_tensor("woT", [SD, D], F32, kind="ExternalInput").ap()
    for name, shape, dt_ in [
        ("lcumT", [P, P], F32),
        ("identb", [P, P], BF16),
        ("ident", [P, P], F32),
        ("ones_col", [P, 1], F32),
        ("ones_row", [1, P], F32),
        ("recip", [P, n_chunks], F32),
        ("r2d", [P, n_chunks], F32),
        ("mask_send", [1, 1], F32),
        ("mask_recv", [1, 1], F32),
    ]:
        io[name] = nc.dram_tensor(name, shape, dt_, kind="ExternalInput").ap()
    io["out"] = nc.dram_tensor("out", [s_local, D], F32, kind="ExternalOutput").ap()

    with tile.TileContext(nc) as tc:
        emit_core_kernel(tc, io, n_chunks, pair_groups)
    nc.compile()
    _BUILD_CACHE[key] = nc
    return nc


def kernel(x, W_update, W_gate, W_out):
    global LAST_EXEC_NS
    import ml_dtypes
    x = np.ascontiguousarray(np.asarray(x, np.float32))
    wgT = np.ascontiguousarray(
        np.asarray(W_gate, np.float32).T.astype(ml_dtypes.bfloat16)
    )
    wuT = np.ascontiguousarray(
        np.asarray(W_update, np.float32).T.astype(ml_dtypes.bfloat16)
    )
    woT = np.ascontiguousarray(np.asarray(W_out, np.float32).T)

    s_local = S // 2
    n_chunks = s_local // P
    nc = build(N_CORES, s_local)

    consts = const_inputs(n_chunks)
    xs = x.reshape(B, 2, s_local, D)
    in_maps = []
    for core in range(N_CORES):
        b, half = core % B, core // B  # cores 0-3 first halves, 4-7 second
        m = {
            "x": np.ascontiguousarray(xs[b, half]),
            "wgT": wgT,
            "wuT": wuT,
            "woT": woT,
            **consts,
            **aux_inputs(core, N_CORES, s_local),
        }
        in_maps.append(m)

    try:
        res = run_bass_kernel_spmd(
            nc, in_maps, core_ids=list(range(N_CORES)), trace=TRACE
        )
    except ModuleNotFoundError:
        # NTFF profile hook unavailable in this environment
        res = run_bass_kernel_spmd(
            nc, in_maps, core_ids=list(range(N_CORES)), trace=False
        )
    LAST_EXEC_NS = res.exec_time_ns

    out = np.empty((B, 2, s_local, D), np.float32)
    for core in range(N_CORES):
        b, half = core % B, core // B
        out[b, half] = res.results[core]["out"]
    return out.reshape(B, S, D)

